# revision 7
# baseline (speedup 1.0000x reference)
"""Trainium2 Bass kernel for nn_DistForecast (RNN + BatchNorm + VAE head).

Math (reference):
    xt[t] = x[:, :, t] @ W_in.T + b_in                    # [B, H]
    h_{t+1} = tanh(xt[t] @ W_hi.T + h_t @ W_hh.T + b_h)   # scan over T
    BN over batch (training stats), then
    y1 = relu(h_bn @ W_hid.T + b_hid)
    out2 = y1 @ W_fc1.T + b_fc1 ; mu, logvar = split(out2)
    sample = eps * (0.5 * exp(logvar)) + relu(mu)
    returns (sample, relu(mu), logvar)

Key algebraic fold: xt[t] @ W_hi.T = x[:,:,t] @ (W_hi @ W_in).T, so the
per-step input projection contracts over N=128 directly (W_eff = W_hi@W_in),
skipping the [T,B,H] intermediate entirely.

Distribution: data-parallel over batch (B=512 -> 64 rows/core on 8 cores);
weights replicated; BatchNorm stats via a 4KB AllReduce.

Device layout per core (SBUF partitions = first dim):
    x_sb   [128n, 64b, 512t]   input, partition = N
    h      [128p, 4j, 64b]     hidden state, h index = j*128+p
    acc    PSUM [128, 4, 64]   pre-activation accumulator (one bank)
Per step: DVE prefills acc with b_eff (broadcast), PE accumulates
4 MMs (W_eff chunk @ x_t) + 16 MMs (W_hh tiles @ h chunks), one ACT
Tanh [128,256] produces the next h.
"""

import os
import sys

for _p in ("/opt/trn_rl_repo", "/opt/trn_rl_repo/concourse"):
    if _p not in sys.path and os.path.isdir(_p):
        sys.path.insert(0, _p)

import numpy as np

import concourse.bass as bass
import concourse.bacc as bacc
import concourse.tile as tile
from concourse import mybir
from concourse.bass_utils import run_bass_kernel_spmd

F32 = mybir.dt.float32
BF16 = mybir.dt.bfloat16
F32R = mybir.dt.float32r
AF = mybir.ActivationFunctionType

# matmul operand dtype mode: "f32", "bf16", or "f32r"
MM_MODE = "f32"

H = 512
N = 128
HC = H // 128  # h chunks
BN_EPS = 1e-5


def _mm(ap, mode):
    """AP for a matmul operand: bitcast f32 storage to f32r when requested."""
    if mode == "f32r":
        return ap.bitcast(F32R)
    return ap


def build_nc(BL, T, n_cores, mode=MM_MODE, x_chunks=8, u_chunk=4,
             act_split=True):
    """Build the SPMD program for one core (BL = local batch).

    u_chunk > 0: compute the input projection u = W_eff @ x_t for u_chunk
    steps per weight-load sweep into dedicated PSUM banks (amortizes the
    W_eff LDWEIGHTS); the per-step accumulator is then seeded by one DVE
    op acc = u[t] + b_eff.  act_split: run tanh as two [128,128] halves so
    the next step's k∈{0,1} matmuls start while the second half finishes.
    """
    w_dt = BF16 if mode == "bf16" else F32
    nc = bacc.Bacc(
        "TRN2", target_bir_lowering=False, debug=False, num_devices=n_cores
    )

    x_in = nc.dram_tensor("x_local", [BL, N, T], F32, kind="ExternalInput").ap()
    epsT_in = nc.dram_tensor("epsT_local", [N, BL], F32, kind="ExternalInput").ap()
    whh_in = nc.dram_tensor("whhT", [128, HC, HC, 128], w_dt, kind="ExternalInput").ap()
    weff_in = nc.dram_tensor("weffT", [128, HC, 128], w_dt, kind="ExternalInput").ap()
    beff_in = nc.dram_tensor("beff", [128, HC], F32, kind="ExternalInput").ap()
    beffb_in = nc.dram_tensor("beff_bcast", [128, HC, BL], F32, kind="ExternalInput").ap()
    whid_in = nc.dram_tensor("whidT", [128, HC, HC, 128], w_dt, kind="ExternalInput").ap()
    bhid_in = nc.dram_tensor("bhid", [128, HC], F32, kind="ExternalInput").ap()
    wfc1_in = nc.dram_tensor("wfc1T", [128, HC, 2, 128], w_dt, kind="ExternalInput").ap()
    bmu_in = nc.dram_tensor("bmu", [128, 1], F32, kind="ExternalInput").ap()
    blv_in = nc.dram_tensor("blv", [128, 2], F32, kind="ExternalInput").ap()
    gamma_in = nc.dram_tensor("gamma_r", [128, HC], F32, kind="ExternalInput").ap()
    beta_in = nc.dram_tensor("beta_r", [128, HC], F32, kind="ExternalInput").ap()

    sampT_out = nc.dram_tensor("sampleT", [N, BL], F32, kind="ExternalOutput").ap()
    muT_out = nc.dram_tensor("muT", [N, BL], F32, kind="ExternalOutput").ap()
    lvT_out = nc.dram_tensor("logvarT", [N, BL], F32, kind="ExternalOutput").ap()

    x_dt = BF16 if mode == "bf16" else F32
    h_dt = BF16 if mode == "bf16" else F32
    B_total = BL * n_cores

    with tile.TileContext(nc) as tc:
        with (
            tc.tile_pool(name="consts", bufs=1) as consts,
            tc.tile_pool(name="xpool", bufs=1) as xpool,
            tc.tile_pool(name="stage", bufs=2) as stage,
            tc.tile_pool(name="hpool", bufs=2) as hpool,
            tc.tile_pool(name="small", bufs=1) as small,
            tc.tile_pool(name="accp", bufs=2, space="PSUM") as accp,
            tc.tile_pool(name="psum2", bufs=1, space="PSUM") as psum2,
            tc.tile_pool(name="dram", bufs=1, space="DRAM") as dram,
        ):
            # ---- weight / const loads ----
            whh = consts.tile([128, HC, HC, 128], w_dt)
            nc.sync.dma_start(out=whh, in_=whh_in)
            weff = consts.tile([128, HC, 128], w_dt)
            nc.sync.dma_start(out=weff, in_=weff_in)
            beff = consts.tile([128, HC], F32)
            nc.sync.dma_start(out=beff, in_=beff_in)
            beffb = consts.tile([128, HC, BL], F32)
            nc.sync.dma_start(out=beffb, in_=beffb_in)
            whid = consts.tile([128, HC, HC, 128], w_dt)
            nc.sync.dma_start(out=whid, in_=whid_in)
            bhid = consts.tile([128, HC], F32)
            nc.sync.dma_start(out=bhid, in_=bhid_in)
            wfc1 = consts.tile([128, HC, 2, 128], w_dt)
            nc.sync.dma_start(out=wfc1, in_=wfc1_in)
            bmu = consts.tile([128, 1], F32)
            nc.sync.dma_start(out=bmu, in_=bmu_in)
            blv = consts.tile([128, 2], F32)
            nc.sync.dma_start(out=blv, in_=blv_in)
            gam = consts.tile([128, HC], F32)
            nc.sync.dma_start(out=gam, in_=gamma_in)
            bet = consts.tile([128, HC], F32)
            nc.sync.dma_start(out=bet, in_=beta_in)
            epsv = consts.tile([128, BL], F32)
            nc.sync.dma_start(out=epsv, in_=epsT_in)

            # ---- x load (t-chunked so the recurrence can start early) ----
            x_sb = xpool.tile([128, BL, T], x_dt)
            tch = T // x_chunks
            for c in range(x_chunks):
                sl = slice(c * tch, (c + 1) * tch)
                src = x_in[:, :, sl].rearrange("b n t -> n b t")
                if x_dt == F32:
                    nc.sync.dma_start(out=x_sb[:, :, sl], in_=src)
                else:
                    xs = stage.tile([128, BL, tch], F32, tag="xstage")
                    nc.sync.dma_start(out=xs, in_=src)
                    nc.vector.tensor_copy(out=x_sb[:, :, sl], in_=xs)

            # ---- recurrence ----
            UC = u_chunk
            if UC:
                upool = tc.tile_pool(name="upool", bufs=2, space="PSUM")
                upool = upool.__enter__()
                n_uch = (T + UC - 1) // UC

                def emit_u_chunk(ci):
                    t0 = ci * UC
                    cl = min(UC, T - t0)
                    u_t = upool.tile([128, HC, UC, BL], F32, tag="u")
                    rx = _mm(x_sb[:, :, t0 : t0 + cl], mode)  # [128, BL, cl]
                    for j in range(HC):
                        # u tile is [p, j, t, b]; bank split between j=1|2.
                        # start on each bank's first write, stop on its last.
                        out_ap = u_t[:, j, 0:cl, :].rearrange("p t b -> p b t")
                        nc.tensor.matmul(
                            out_ap,
                            lhsT=_mm(weff[:, j, :], mode),
                            rhs=rx,
                            start=(j % 2 == 0),
                            stop=(j % 2 == 1),
                        )
                    return u_t

                u_tiles = {0: emit_u_chunk(0)}

            h = hpool.tile([128, HC, BL], h_dt, tag="h")
            nc.vector.memset(h, 0.0)
            for t in range(T):
                if UC:
                    ci = t // UC
                    if t % UC == 0 and ci + 1 < n_uch:
                        u_tiles[ci + 1] = emit_u_chunk(ci + 1)
                        u_tiles.pop(ci - 1, None)
                    u_cur = u_tiles[ci][:, :, t % UC, :]  # [128, HC, BL]
                h_new = hpool.tile([128, HC, BL], h_dt, tag="h")

                if UC and t == 0:
                    # h1 = tanh(u0 + b_eff): read u psum directly, no acc.
                    for j in range(HC):
                        nc.scalar.activation(
                            h_new[:, j, :], u_cur[:, j, :], AF.Tanh,
                            bias=beff[:, j : j + 1],
                        )
                    h = h_new
                    continue

                acc = accp.tile([128, HC, BL], F32, tag="acc")
                # First use of each of the two acc slots: open a real
                # accumulation group (start=True zero-arms the bank); bias
                # and u are added afterwards by DVE + per-chunk ACT bias.
                # From t>=2 every bank byte was matmul-written last
                # iteration, so a DVE prefill of u[t]+b_eff survives and
                # every MM accumulates (skip_group_check: sim-only).
                fresh = t < (2 if not UC else 3)
                skip = not fresh
                if not fresh:
                    if UC:
                        nc.vector.tensor_add(acc, u_cur, beffb)
                    else:
                        nc.vector.tensor_copy(out=acc, in_=beffb)
                if not UC:
                    for j in range(HC):
                        nc.tensor.matmul(
                            acc[:, j, :],
                            lhsT=_mm(weff[:, j, :], mode),
                            rhs=_mm(x_sb[:, :, t], mode),
                            start=(fresh and j == 0),
                            stop=(t == 0 and j == HC - 1),
                            skip_group_check=skip,
                        )
                if t > 0:
                    for k in range(HC):
                        rh = _mm(h[:, k, :], mode)
                        for j in range(HC):
                            nc.tensor.matmul(
                                acc[:, j, :],
                                lhsT=_mm(whh[:, k, j, :], mode),
                                rhs=rh,
                                start=(UC and fresh and k == 0 and j == 0),
                                stop=((k, j) == (HC - 1, HC - 1)),
                                skip_group_check=skip,
                            )
                if UC and fresh and t > 0:
                    # acc holds only W_hh @ h; add u[t] before the tanh.
                    nc.vector.tensor_add(acc, acc, u_cur)
                if fresh:
                    for j in range(HC):
                        nc.scalar.activation(
                            h_new[:, j, :], acc[:, j, :], AF.Tanh,
                            bias=beff[:, j : j + 1],
                        )
                elif act_split:
                    half = HC // 2
                    nc.scalar.activation(h_new[:, :half, :],
                                         acc[:, :half, :], AF.Tanh)
                    nc.scalar.activation(h_new[:, half:, :],
                                         acc[:, half:, :], AF.Tanh)
                else:
                    nc.scalar.activation(h_new[:, :, :], acc[:, :, :], AF.Tanh)
                h = h_new
            if UC:
                upool.release()

            # ---- BatchNorm stats + AllReduce ----
            stats = small.tile([128, 2, HC], F32)
            nc.vector.reduce_sum(out=stats[:, 0, :], in_=h, axis=mybir.AxisListType.X)
            hsq = small.tile([128, HC, BL], F32)
            nc.vector.tensor_mul(hsq, h, h)
            nc.vector.reduce_sum(out=stats[:, 1, :], in_=hsq, axis=mybir.AxisListType.X)

            cc_in = dram.tile([128, 2 * HC], F32)
            cc_out = dram.tile([128, 2 * HC], F32)
            nc.sync.dma_start(out=cc_in, in_=stats)
            if n_cores > 1:
                nc.gpsimd.collective_compute(
                    "AllReduce",
                    mybir.AluOpType.add,
                    ins=[cc_in.opt()],
                    outs=[cc_out.opt()],
                    replica_groups=[list(range(n_cores))],
                )
                gstats = small.tile([128, 2, HC], F32)
                nc.sync.dma_start(out=gstats, in_=cc_out)
            else:
                gstats = stats

            mean = small.tile([128, HC], F32)
            nc.vector.tensor_scalar_mul(mean, gstats[:, 0, :], 1.0 / B_total)
            ex2 = small.tile([128, HC], F32)
            nc.vector.tensor_scalar_mul(ex2, gstats[:, 1, :], 1.0 / B_total)
            var = small.tile([128, HC], F32)
            nc.vector.tensor_mul(var, mean, mean)
            nc.vector.tensor_sub(var, ex2, var)
            epsc = small.tile([128, 1], F32)
            nc.vector.memset(epsc, float(BN_EPS))
            sd = small.tile([128, HC], F32)
            nc.scalar.activation(sd, var, AF.Sqrt, bias=epsc[:, 0:1])
            istd = small.tile([128, HC], F32)
            nc.vector.reciprocal(istd, sd)
            gs = small.tile([128, HC], F32)  # gamma / std
            nc.vector.tensor_mul(gs, gam, istd)
            bs = small.tile([128, HC], F32)  # beta - mean * gamma / std
            nc.vector.tensor_mul(bs, mean, gs)
            nc.vector.tensor_sub(bs, bet, bs)

            hbn = small.tile([128, HC, BL], h_dt)
            for j in range(HC):
                nc.scalar.activation(
                    hbn[:, j, :], h[:, j, :], AF.Identity,
                    bias=bs[:, j : j + 1], scale=gs[:, j : j + 1],
                )

            # ---- head ----
            p1 = psum2.tile([128, HC, BL], F32, tag="p1")
            for k in range(HC):
                rk = _mm(hbn[:, k, :], mode)
                for j in range(HC):
                    nc.tensor.matmul(
                        p1[:, j, :], lhsT=_mm(whid[:, k, j, :], mode), rhs=rk,
                        start=(k == 0 and j == 0),
                        stop=(k == HC - 1 and j == HC - 1),
                    )
            y1 = small.tile([128, HC, BL], h_dt)
            for j in range(HC):
                nc.scalar.activation(
                    y1[:, j, :], p1[:, j, :], AF.Relu, bias=bhid[:, j : j + 1]
                )

            p2 = psum2.tile([128, 2, BL], F32, tag="p2")
            for k in range(HC):
                rk = _mm(y1[:, k, :], mode)
                for m in range(2):
                    nc.tensor.matmul(
                        p2[:, m, :], lhsT=_mm(wfc1[:, k, m, :], mode), rhs=rk,
                        start=(k == 0 and m == 0),
                        stop=(k == HC - 1 and m == 1),
                    )

            mu = small.tile([128, BL], F32)
            nc.scalar.activation(mu, p2[:, 0, :], AF.Relu, bias=bmu[:, 0:1])
            lv = small.tile([128, BL], F32)
            nc.scalar.activation(lv, p2[:, 1, :], AF.Identity, bias=blv[:, 0:1])
            sdv = small.tile([128, BL], F32)
            nc.scalar.activation(sdv, p2[:, 1, :], AF.Exp, bias=blv[:, 1:2])
            samp = small.tile([128, BL], F32)
            nc.vector.tensor_mul(samp, epsv, sdv)
            nc.vector.tensor_add(samp, samp, mu)

            nc.sync.dma_start(out=sampT_out, in_=samp)
            nc.sync.dma_start(out=muT_out, in_=mu)
            nc.sync.dma_start(out=lvT_out, in_=lv)

    nc.compile()
    return nc


def host_prep(x, eps, W_in, b_in, W_h, b_h, gamma, beta, W_hid, b_hid,
              W_fc1, b_fc1, BL, n_cores, mode=MM_MODE):
    """Host-side weight folding + per-core input slicing."""
    import ml_dtypes

    w_np = ml_dtypes.bfloat16 if mode == "bf16" else np.float32

    x = np.asarray(x, np.float32)
    eps = np.asarray(eps, np.float32)
    W_in = np.asarray(W_in, np.float32)
    b_in = np.asarray(b_in, np.float32)
    W_h = np.asarray(W_h, np.float32)
    b_h = np.asarray(b_h, np.float32)
    gamma = np.asarray(gamma, np.float32)
    beta = np.asarray(beta, np.float32)
    W_hid = np.asarray(W_hid, np.float32)
    b_hid = np.asarray(b_hid, np.float32)
    W_fc1 = np.asarray(W_fc1, np.float32)
    b_fc1 = np.asarray(b_fc1, np.float32)

    W_hi = W_h[:, :H]
    W_hh = W_h[:, H:]
    W_eff = W_hi @ W_in            # [H, N]
    b_eff = W_hi @ b_in + b_h      # [H]

    def tiles4(WT, mch):  # WT: [K, M] -> [128, KC, mch, 128]
        K, M = WT.shape
        return np.ascontiguousarray(
            WT.reshape(K // 128, 128, mch, 128).transpose(1, 0, 2, 3)
        ).astype(w_np)

    whhT = tiles4(W_hh.T, HC)            # [k',k,j,j']
    whidT = tiles4(W_hid.T, HC)
    wfc1T = tiles4(W_fc1.T, 2)
    weffT = np.ascontiguousarray(W_eff.T.reshape(128, HC, 128)).astype(w_np)
    beff = np.ascontiguousarray(b_eff.reshape(HC, 128).T)
    beffb = np.ascontiguousarray(
        np.broadcast_to(beff[:, :, None], (128, HC, BL))
    ).astype(np.float32)
    bhid = np.ascontiguousarray(b_hid.reshape(HC, 128).T)
    bmu = np.ascontiguousarray(b_fc1[:N].reshape(N, 1))
    blv = np.ascontiguousarray(
        np.stack([b_fc1[N:], b_fc1[N:] + np.log(0.5).astype(np.float32)], axis=1)
    ).astype(np.float32)
    gam = np.ascontiguousarray(gamma.reshape(HC, 128).T)
    bet = np.ascontiguousarray(beta.reshape(HC, 128).T)
    epsT = np.ascontiguousarray(eps.T)   # [N, B]

    shared = dict(
        whhT=whhT, weffT=weffT, beff=beff, beff_bcast=beffb, whidT=whidT,
        bhid=bhid, wfc1T=wfc1T, bmu=bmu, blv=blv, gamma_r=gam, beta_r=bet,
    )
    in_maps = []
    for c in range(n_cores):
        m = dict(shared)
        m["x_local"] = np.ascontiguousarray(x[c * BL : (c + 1) * BL])
        m["epsT_local"] = np.ascontiguousarray(epsT[:, c * BL : (c + 1) * BL])
        in_maps.append(m)
    return in_maps


def assemble(results):
    """Gather per-core [N, BL] outputs into full [B, N] arrays."""
    samp = np.concatenate([r["sampleT"] for r in results], axis=1).T
    mu = np.concatenate([r["muT"] for r in results], axis=1).T
    lv = np.concatenate([r["logvarT"] for r in results], axis=1).T
    return (np.ascontiguousarray(samp), np.ascontiguousarray(mu),
            np.ascontiguousarray(lv))


_NC_CACHE = {}


def kernel(x, eps, W_in, b_in, W_h, b_h, gamma, beta, W_hid, b_hid,
           W_fc1, b_fc1, trace=False, mode=MM_MODE):
    B, n, T = x.shape
    n_cores = 8
    BL = B // n_cores
    key = (BL, T, n_cores, mode)
    if key not in _NC_CACHE:
        _NC_CACHE[key] = build_nc(BL, T, n_cores, mode=mode)
    nc = _NC_CACHE[key]
    in_maps = host_prep(x, eps, W_in, b_in, W_h, b_h, gamma, beta, W_hid,
                        b_hid, W_fc1, b_fc1, BL, n_cores, mode=mode)
    res = run_bass_kernel_spmd(nc, in_maps, core_ids=list(range(n_cores)),
                               trace=trace)
    out = assemble(res.results)
    if trace:
        return out, res
    return out


# revision 17
# speedup vs baseline: 1.8484x; 1.8484x over previous
"""Trainium2 Bass kernel for nn_DistForecast (RNN + BatchNorm + VAE head).

Math (reference):
    xt[t] = x[:, :, t] @ W_in.T + b_in                    # [B, H]
    h_{t+1} = tanh(xt[t] @ W_hi.T + h_t @ W_hh.T + b_h)   # scan over T
    BN over batch (training stats), then
    y1 = relu(h_bn @ W_hid.T + b_hid)
    out2 = y1 @ W_fc1.T + b_fc1 ; mu, logvar = split(out2)
    sample = eps * (0.5 * exp(logvar)) + relu(mu)
    returns (sample, relu(mu), logvar)

Key algebraic fold: xt[t] @ W_hi.T = x[:,:,t] @ (W_hi @ W_in).T, so the
per-step input projection contracts over N=128 directly (W_eff = W_hi@W_in),
skipping the [T,B,H] intermediate entirely.

Distribution: data-parallel over batch (B=512 -> 64 rows/core on 8 cores);
weights replicated; BatchNorm stats via a 4KB AllReduce.

Device layout per core (SBUF partitions = first dim):
    x_sb   [128n, 64b, 512t]   input, partition = N
    h      [128p, 4j, 64b]     hidden state, h index = j*128+p
    acc    PSUM [128, 4, 64]   pre-activation accumulator (one bank)
Per step: DVE prefills acc with b_eff (broadcast), PE accumulates
4 MMs (W_eff chunk @ x_t) + 16 MMs (W_hh tiles @ h chunks), one ACT
Tanh [128,256] produces the next h.
"""

import os
import sys

for _p in ("/opt/trn_rl_repo", "/opt/trn_rl_repo/concourse"):
    if _p not in sys.path and os.path.isdir(_p):
        sys.path.insert(0, _p)

import numpy as np

import concourse.bass as bass
import concourse.bacc as bacc
import concourse.tile as tile
from concourse import mybir
from concourse.bass_utils import run_bass_kernel_spmd

F32 = mybir.dt.float32
BF16 = mybir.dt.bfloat16
F32R = mybir.dt.float32r
AF = mybir.ActivationFunctionType

# matmul operand dtype mode: "f32", "bf16", or "f32r"
MM_MODE = "bf16"

H = 512
N = 128
HC = H // 128  # h chunks
BN_EPS = 1e-5


def _mm(ap, mode):
    """AP for a matmul operand: bitcast f32 storage to f32r when requested."""
    if mode == "f32r":
        return ap.bitcast(F32R)
    return ap


def build_nc(BL, T, n_cores, mode=MM_MODE, x_chunks=8, u_chunk=4,
             act_split=True, repeat=1):
    """Build the SPMD program for one core (BL = local batch).

    u_chunk > 0: compute the input projection u = W_eff @ x_t for u_chunk
    steps per weight-load sweep into dedicated PSUM banks (amortizes the
    W_eff LDWEIGHTS); the per-step accumulator is then seeded by one DVE
    op acc = u[t] + b_eff.  act_split: run tanh as two [128,128] halves so
    the next step's k∈{0,1} matmuls start while the second half finishes.
    """
    w_dt = BF16 if mode == "bf16" else F32
    nc = bacc.Bacc(
        "TRN2", target_bir_lowering=False, debug=False, num_devices=n_cores
    )

    x_in = nc.dram_tensor("x_local", [BL, N, T], F32, kind="ExternalInput").ap()
    epsT_in = nc.dram_tensor("epsT_local", [N, BL], F32, kind="ExternalInput").ap()
    whh_in = nc.dram_tensor("whhT", [128, HC, HC, 128], w_dt, kind="ExternalInput").ap()
    weff_in = nc.dram_tensor("weffT", [128, HC, 128], w_dt, kind="ExternalInput").ap()
    beff_in = nc.dram_tensor("beff", [128, HC], F32, kind="ExternalInput").ap()
    beffb_in = nc.dram_tensor("beff_bcast", [128, HC, BL], F32, kind="ExternalInput").ap()
    whid_in = nc.dram_tensor("whidT", [128, HC, HC, 128], w_dt, kind="ExternalInput").ap()
    bhid_in = nc.dram_tensor("bhid", [128, HC], F32, kind="ExternalInput").ap()
    wfc1_in = nc.dram_tensor("wfc1T", [128, HC, 2, 128], w_dt, kind="ExternalInput").ap()
    bmu_in = nc.dram_tensor("bmu", [128, 1], F32, kind="ExternalInput").ap()
    blv_in = nc.dram_tensor("blv", [128, 2], F32, kind="ExternalInput").ap()
    gamma_in = nc.dram_tensor("gamma_r", [128, HC], F32, kind="ExternalInput").ap()
    beta_in = nc.dram_tensor("beta_r", [128, HC], F32, kind="ExternalInput").ap()

    sampT_out = nc.dram_tensor("sampleT", [N, BL], F32, kind="ExternalOutput").ap()
    muT_out = nc.dram_tensor("muT", [N, BL], F32, kind="ExternalOutput").ap()
    lvT_out = nc.dram_tensor("logvarT", [N, BL], F32, kind="ExternalOutput").ap()

    x_dt = BF16 if mode == "bf16" else F32
    h_dt = BF16 if mode == "bf16" else F32
    B_total = BL * n_cores

    with tile.TileContext(nc) as tc:
        with (
            tc.tile_pool(name="consts", bufs=1) as consts,
            tc.tile_pool(name="xpool", bufs=1) as xpool,
            tc.tile_pool(name="stage", bufs=2) as stage,
            tc.tile_pool(name="hpool", bufs=2) as hpool,
            tc.tile_pool(name="small", bufs=1) as small,
            tc.tile_pool(name="accp", bufs=2, space="PSUM") as accp,
            tc.tile_pool(name="psum2", bufs=1, space="PSUM") as psum2,
            tc.tile_pool(name="dram", bufs=1, space="DRAM") as dram,
        ):
            # ---- weight / const loads ----
            whh = consts.tile([128, HC, HC, 128], w_dt)
            nc.sync.dma_start(out=whh, in_=whh_in)
            weff = consts.tile([128, HC, 128], w_dt)
            nc.sync.dma_start(out=weff, in_=weff_in)
            beff = consts.tile([128, HC], F32)
            nc.sync.dma_start(out=beff, in_=beff_in)
            beffb = consts.tile([128, HC, BL], F32)
            nc.sync.dma_start(out=beffb, in_=beffb_in)
            whid = consts.tile([128, HC, HC, 128], w_dt)
            nc.sync.dma_start(out=whid, in_=whid_in)
            bhid = consts.tile([128, HC], F32)
            nc.sync.dma_start(out=bhid, in_=bhid_in)
            wfc1 = consts.tile([128, HC, 2, 128], w_dt)
            nc.sync.dma_start(out=wfc1, in_=wfc1_in)
            bmu = consts.tile([128, 1], F32)
            nc.sync.dma_start(out=bmu, in_=bmu_in)
            blv = consts.tile([128, 2], F32)
            nc.sync.dma_start(out=blv, in_=blv_in)
            gam = consts.tile([128, HC], F32)
            nc.sync.dma_start(out=gam, in_=gamma_in)
            bet = consts.tile([128, HC], F32)
            nc.sync.dma_start(out=bet, in_=beta_in)
            epsv = consts.tile([128, BL], F32)
            nc.sync.dma_start(out=epsv, in_=epsT_in)

            # ---- x load (t-chunked so the recurrence can start early) ----
            x_sb = xpool.tile([128, BL, T], x_dt)
            tch = T // x_chunks
            for c in range(x_chunks):
                sl = slice(c * tch, (c + 1) * tch)
                src = x_in[:, :, sl].rearrange("b n t -> n b t")
                if x_dt == F32:
                    nc.sync.dma_start(out=x_sb[:, :, sl], in_=src)
                else:
                    xs = stage.tile([128, BL, tch], F32, tag="xstage")
                    nc.sync.dma_start(out=xs, in_=src)
                    nc.vector.tensor_copy(out=x_sb[:, :, sl], in_=xs)

            # ---- recurrence ----
            UC = u_chunk
            if UC:
                assert T % UC == 0, "u_chunk must divide T"
                upool = tc.alloc_tile_pool(name="upool", bufs=2, space="PSUM")
                n_uch = T // UC

                def emit_u_chunk(ci):
                    t0 = ci * UC
                    u_t = upool.tile([128, HC, BL, UC], F32, tag="u")
                    rx = _mm(x_sb[:, :, t0 : t0 + UC], mode)  # [128, BL, UC]
                    for j in range(HC):
                        # u tile is [p, j, b, t]; bank split between j=1|2.
                        # start on each bank's first write, stop on its last.
                        out_ap = u_t[:, j, :, :].rearrange("p b t -> p (b t)")
                        nc.tensor.matmul(
                            out_ap,
                            lhsT=_mm(weff[:, j, :], mode),
                            rhs=rx,
                            start=(j % 2 == 0),
                            stop=(j % 2 == 1),
                        )
                    return u_t

            for rep in range(repeat):  # repeat>1: timing-only builds
              if UC:
                u_tiles = {0: emit_u_chunk(0)}
              h = hpool.tile([128, HC, BL], h_dt, tag="h")
              nc.vector.memset(h, 0.0)
              for t in range(T):
                if UC:
                    ci = t // UC
                    if t % UC == 0 and ci + 1 < n_uch:
                        u_tiles[ci + 1] = emit_u_chunk(ci + 1)
                        u_tiles.pop(ci - 1, None)
                    u_cur = u_tiles[ci][:, :, :, t % UC]  # [128, HC, BL]
                h_new = hpool.tile([128, HC, BL], h_dt, tag="h")

                if UC and t == 0:
                    # h1 = tanh(u0 + b_eff): read u psum directly, no acc.
                    for j in range(HC):
                        nc.scalar.activation(
                            h_new[:, j, :], u_cur[:, j, :], AF.Tanh,
                            bias=beff[:, j : j + 1],
                        )
                    h = h_new
                    continue

                acc = accp.tile([128, HC, BL], F32, tag="acc")
                # First use of each of the two acc slots: open a real
                # accumulation group (start=True zero-arms the bank); bias
                # and u are added afterwards by DVE + per-chunk ACT bias.
                # From t>=2 every bank byte was matmul-written last
                # iteration, so a DVE prefill of u[t]+b_eff survives and
                # every MM accumulates (skip_group_check: sim-only).
                fresh = rep == 0 and t < (2 if not UC else 3)
                skip = not fresh
                if not fresh:
                    if UC:
                        # acc = u[t] + b_eff (one PSUM input, one SBUF input)
                        nc.vector.tensor_add(acc, u_cur, beffb)
                    else:
                        nc.vector.tensor_copy(out=acc, in_=beffb)
                if not UC or fresh:
                    # direct per-step input-projection MMs into acc
                    for j in range(HC):
                        nc.tensor.matmul(
                            acc[:, j, :],
                            lhsT=_mm(weff[:, j, :], mode),
                            rhs=_mm(x_sb[:, :, t], mode),
                            start=(fresh and j == 0),
                            stop=(t == 0 and j == HC - 1),
                            skip_group_check=skip,
                        )
                if t > 0:
                    for k in range(HC):
                        rh = _mm(h[:, k, :], mode)
                        for j in range(HC):
                            nc.tensor.matmul(
                                acc[:, j, :],
                                lhsT=_mm(whh[:, k, j, :], mode),
                                rhs=rh,
                                start=False,
                                stop=((k, j) == (HC - 1, HC - 1)),
                                skip_group_check=skip,
                            )
                if fresh:
                    for j in range(HC):
                        nc.scalar.activation(
                            h_new[:, j, :], acc[:, j, :], AF.Tanh,
                            bias=beff[:, j : j + 1],
                        )
                elif act_split:
                    half = HC // 2
                    nc.scalar.activation(h_new[:, :half, :],
                                         acc[:, :half, :], AF.Tanh)
                    nc.scalar.activation(h_new[:, half:, :],
                                         acc[:, half:, :], AF.Tanh)
                else:
                    nc.scalar.activation(h_new[:, :, :], acc[:, :, :], AF.Tanh)
                h = h_new
            if UC:
                upool.release()

            # ---- BatchNorm stats + AllReduce ----
            stats = small.tile([128, 2, HC], F32)
            nc.vector.reduce_sum(out=stats[:, 0, :], in_=h, axis=mybir.AxisListType.X)
            hsq = small.tile([128, HC, BL], F32)
            nc.vector.tensor_mul(hsq, h, h)
            nc.vector.reduce_sum(out=stats[:, 1, :], in_=hsq, axis=mybir.AxisListType.X)

            cc_in = dram.tile([128, 2 * HC], F32)
            cc_out = dram.tile([128, 2 * HC], F32)
            nc.sync.dma_start(out=cc_in, in_=stats)
            if n_cores > 1:
                nc.gpsimd.collective_compute(
                    "AllReduce",
                    mybir.AluOpType.add,
                    ins=[cc_in.opt()],
                    outs=[cc_out.opt()],
                    replica_groups=[list(range(n_cores))],
                )
                gstats = small.tile([128, 2, HC], F32)
                nc.sync.dma_start(out=gstats, in_=cc_out)
            else:
                gstats = stats

            mean = small.tile([128, HC], F32)
            nc.vector.tensor_scalar_mul(mean, gstats[:, 0, :], 1.0 / B_total)
            ex2 = small.tile([128, HC], F32)
            nc.vector.tensor_scalar_mul(ex2, gstats[:, 1, :], 1.0 / B_total)
            var = small.tile([128, HC], F32)
            nc.vector.tensor_mul(var, mean, mean)
            nc.vector.tensor_sub(var, ex2, var)
            epsc = small.tile([128, 1], F32)
            nc.vector.memset(epsc, float(BN_EPS))
            sd = small.tile([128, HC], F32)
            nc.scalar.activation(sd, var, AF.Sqrt, bias=epsc[:, 0:1])
            istd = small.tile([128, HC], F32)
            nc.vector.reciprocal(istd, sd)
            gs = small.tile([128, HC], F32)  # gamma / std
            nc.vector.tensor_mul(gs, gam, istd)
            bs = small.tile([128, HC], F32)  # beta - mean * gamma / std
            nc.vector.tensor_mul(bs, mean, gs)
            nc.vector.tensor_sub(bs, bet, bs)

            hbn = small.tile([128, HC, BL], h_dt)
            for j in range(HC):
                nc.scalar.activation(
                    hbn[:, j, :], h[:, j, :], AF.Identity,
                    bias=bs[:, j : j + 1], scale=gs[:, j : j + 1],
                )

            # ---- head ----
            p1 = psum2.tile([128, HC, BL], F32, tag="p1")
            for k in range(HC):
                rk = _mm(hbn[:, k, :], mode)
                for j in range(HC):
                    nc.tensor.matmul(
                        p1[:, j, :], lhsT=_mm(whid[:, k, j, :], mode), rhs=rk,
                        start=(k == 0 and j == 0),
                        stop=(k == HC - 1 and j == HC - 1),
                    )
            y1 = small.tile([128, HC, BL], h_dt)
            for j in range(HC):
                nc.scalar.activation(
                    y1[:, j, :], p1[:, j, :], AF.Relu, bias=bhid[:, j : j + 1]
                )

            p2 = psum2.tile([128, 2, BL], F32, tag="p2")
            for k in range(HC):
                rk = _mm(y1[:, k, :], mode)
                for m in range(2):
                    nc.tensor.matmul(
                        p2[:, m, :], lhsT=_mm(wfc1[:, k, m, :], mode), rhs=rk,
                        start=(k == 0 and m == 0),
                        stop=(k == HC - 1 and m == 1),
                    )

            mu = small.tile([128, BL], F32)
            nc.scalar.activation(mu, p2[:, 0, :], AF.Relu, bias=bmu[:, 0:1])
            lv = small.tile([128, BL], F32)
            nc.scalar.activation(lv, p2[:, 1, :], AF.Identity, bias=blv[:, 0:1])
            sdv = small.tile([128, BL], F32)
            nc.scalar.activation(sdv, p2[:, 1, :], AF.Exp, bias=blv[:, 1:2])
            samp = small.tile([128, BL], F32)
            nc.vector.tensor_mul(samp, epsv, sdv)
            nc.vector.tensor_add(samp, samp, mu)

            nc.sync.dma_start(out=sampT_out, in_=samp)
            nc.sync.dma_start(out=muT_out, in_=mu)
            nc.sync.dma_start(out=lvT_out, in_=lv)

    nc.compile()
    return nc


def host_prep(x, eps, W_in, b_in, W_h, b_h, gamma, beta, W_hid, b_hid,
              W_fc1, b_fc1, BL, n_cores, mode=MM_MODE):
    """Host-side weight folding + per-core input slicing."""
    import ml_dtypes

    w_np = ml_dtypes.bfloat16 if mode == "bf16" else np.float32

    x = np.asarray(x, np.float32)
    eps = np.asarray(eps, np.float32)
    W_in = np.asarray(W_in, np.float32)
    b_in = np.asarray(b_in, np.float32)
    W_h = np.asarray(W_h, np.float32)
    b_h = np.asarray(b_h, np.float32)
    gamma = np.asarray(gamma, np.float32)
    beta = np.asarray(beta, np.float32)
    W_hid = np.asarray(W_hid, np.float32)
    b_hid = np.asarray(b_hid, np.float32)
    W_fc1 = np.asarray(W_fc1, np.float32)
    b_fc1 = np.asarray(b_fc1, np.float32)

    W_hi = W_h[:, :H]
    W_hh = W_h[:, H:]
    W_eff = W_hi @ W_in            # [H, N]
    b_eff = W_hi @ b_in + b_h      # [H]

    def tiles4(WT, mch):  # WT: [K, M] -> [128, KC, mch, 128]
        K, M = WT.shape
        return np.ascontiguousarray(
            WT.reshape(K // 128, 128, mch, 128).transpose(1, 0, 2, 3)
        ).astype(w_np)

    whhT = tiles4(W_hh.T, HC)            # [k',k,j,j']
    whidT = tiles4(W_hid.T, HC)
    wfc1T = tiles4(W_fc1.T, 2)
    weffT = np.ascontiguousarray(W_eff.T.reshape(128, HC, 128)).astype(w_np)
    beff = np.ascontiguousarray(b_eff.reshape(HC, 128).T)
    beffb = np.ascontiguousarray(
        np.broadcast_to(beff[:, :, None], (128, HC, BL))
    ).astype(np.float32)
    bhid = np.ascontiguousarray(b_hid.reshape(HC, 128).T)
    bmu = np.ascontiguousarray(b_fc1[:N].reshape(N, 1))
    blv = np.ascontiguousarray(
        np.stack([b_fc1[N:], b_fc1[N:] + np.log(0.5).astype(np.float32)], axis=1)
    ).astype(np.float32)
    gam = np.ascontiguousarray(gamma.reshape(HC, 128).T)
    bet = np.ascontiguousarray(beta.reshape(HC, 128).T)
    epsT = np.ascontiguousarray(eps.T)   # [N, B]

    shared = dict(
        whhT=whhT, weffT=weffT, beff=beff, beff_bcast=beffb, whidT=whidT,
        bhid=bhid, wfc1T=wfc1T, bmu=bmu, blv=blv, gamma_r=gam, beta_r=bet,
    )
    in_maps = []
    for c in range(n_cores):
        m = dict(shared)
        m["x_local"] = np.ascontiguousarray(x[c * BL : (c + 1) * BL])
        m["epsT_local"] = np.ascontiguousarray(epsT[:, c * BL : (c + 1) * BL])
        in_maps.append(m)
    return in_maps


def assemble(results):
    """Gather per-core [N, BL] outputs into full [B, N] arrays."""
    samp = np.concatenate([r["sampleT"] for r in results], axis=1).T
    mu = np.concatenate([r["muT"] for r in results], axis=1).T
    lv = np.concatenate([r["logvarT"] for r in results], axis=1).T
    return (np.ascontiguousarray(samp), np.ascontiguousarray(mu),
            np.ascontiguousarray(lv))


_NC_CACHE = {}


def _run_once(x, eps, W_in, b_in, W_h, b_h, gamma, beta, W_hid, b_hid,
              W_fc1, b_fc1, mode=MM_MODE):
    B, n, T = x.shape
    n_cores = 8
    BL = B // n_cores
    key = (BL, T, n_cores, mode)
    if key not in _NC_CACHE:
        _NC_CACHE[key] = build_nc(BL, T, n_cores, mode=mode)
    nc = _NC_CACHE[key]
    in_maps = host_prep(x, eps, W_in, b_in, W_h, b_h, gamma, beta, W_hid,
                        b_hid, W_fc1, b_fc1, BL, n_cores, mode=mode)
    res = run_bass_kernel_spmd(nc, in_maps, core_ids=list(range(n_cores)))
    return assemble(res.results)


_CHILD_CODE = """
import sys
sys.path.insert(0, {moddir!r})
import numpy as np
import kernel
d = np.load({inp!r})
out = kernel._run_once(**{{k: d[k] for k in d.files}})
np.savez({outp!r}, sample=out[0], mu_r=out[1], logvar=out[2])
"""


def _run_subprocess(inputs):
    """Run the device execution in a fresh process (recovers from a dead
    axon backend in this process)."""
    import subprocess
    import tempfile

    moddir = os.path.dirname(os.path.abspath(__file__))
    with tempfile.TemporaryDirectory() as td:
        inp = os.path.join(td, "in.npz")
        outp = os.path.join(td, "out.npz")
        np.savez(inp, **inputs)
        code = _CHILD_CODE.format(moddir=moddir, inp=inp, outp=outp)
        subprocess.run([sys.executable, "-c", code], check=True, timeout=3600)
        d = np.load(outp)
        return (d["sample"], d["mu_r"], d["logvar"])


def kernel(x, eps, W_in, b_in, W_h, b_h, gamma, beta, W_hid, b_hid,
           W_fc1, b_fc1):
    """Full-input entry point: returns (sample, mu_r, logvar) as np arrays.

    Retries in a fresh subprocess on transient device failures (rare
    NRT_EXEC_UNIT_UNRECOVERABLE / axon hangups observed on first execs)."""
    inputs = dict(x=x, eps=eps, W_in=W_in, b_in=b_in, W_h=W_h, b_h=b_h,
                  gamma=gamma, beta=beta, W_hid=W_hid, b_hid=b_hid,
                  W_fc1=W_fc1, b_fc1=b_fc1)
    try:
        return _run_once(**inputs)
    except Exception as e:
        sys.stderr.write(f"kernel: in-process exec failed ({type(e).__name__}:"
                         f" {e}); retrying in subprocess\n")
    last = None
    for _ in range(2):
        try:
            return _run_subprocess(inputs)
        except Exception as e:  # noqa: PERF203
            last = e
            sys.stderr.write(f"kernel: subprocess retry failed "
                             f"({type(e).__name__}: {e})\n")
    raise last
